# revision 3
# baseline (speedup 1.0000x reference)
"""Trainium2 Bass kernel for the AutoRegressiveLSTM problem — v3.

Data-parallel over batch (512 -> 64 rows/core, 8 cores). Design:

- Gate matmuls are fp8-e4m3 DoubleRow pairs (two 128-unit k-tiles per
  instruction) in FULL-ARRAY tile mode (128,128). The two gate-column halves
  land in one folded [128, 512] PSUM tile via shifted stationaries:
  [h|0] writes partitions 0:64 (cols off..off+512), [0|h] writes partitions
  64:128 (cols off+512..off+1024). ~39 TMAC/s measured, and every PE
  instruction (gates, transposes, x/pred terms, dense) shares ONE tile mode
  -> no TensorE mode-switch drains.
- Transposes are full-width [128,128] (4 per cell), each producing the
  (k, k+4) plane pair the DoubleRow stationaries consume.
- x / pred-feedback terms are bf16 (accuracy) with the fp8 scale folded
  into W1 (x @ (W1*S)) and undone for free by the activation scale.
- fp8 scales: h*128, U-weights*256 -> z in PSUM scaled by 32768;
  activation(scale=1/32768) folds it out exactly (powers of 2).
- The tail's cross-engine latency chain is sliced: chunk 0 (128 cols) runs
  tanh(c)/sigmoid(o)/h-mul/transpose/fp8-copies first so the next cell's
  first DoubleRow pair starts ~1.5us after the gate stream ends, inside the
  x-prestart cover window; chunks 1-3 follow while pair 0 streams.
- Anti-HAM dummy transposes keep the PE activity monitor fed through the
  AR tails (no x-term to cover there); without them the clock-gate halves
  the clock for ~3.4us after every idle window.

Max-rel error vs fp32 reference ~1.1e-2 on hardware (tolerance 2e-2).
"""

from contextlib import ExitStack

import numpy as np
import ml_dtypes

import concourse.bass as bass
import concourse.tile as tile
from concourse import bacc, mybir
from concourse.bass_utils import run_bass_kernel_spmd

BF16 = mybir.dt.bfloat16
F8 = mybir.dt.float8e4
F32 = mybir.dt.float32
AF = mybir.ActivationFunctionType
DR = mybir.MatmulPerfMode.DoubleRow

NCORES = 8
B_FULL = 512
BS = B_FULL // NCORES   # 64 batch rows per core
T = 64                  # warmup sequence length
F = 128                 # features
U = 1024                # LSTM units
G = 4 * U               # 4096 gate columns
NP = 4                  # k-tile pairs (j, j+4)
OUT_STEPS = 32
EPS = 1e-7
H = 512                 # half-gate width
GATE_OFF = {"i": 0, "f": U, "g": 2 * U, "o": 3 * U}
GORDER = ("f", "i", "g", "o")  # f first: the c-update chain hangs off f/i/g

S_H = 128.0             # h quantization scale
S_W = 256.0             # U1 / (W2+U2) quantization scale
S_Z = S_H * S_W         # 32768; z lands in PSUM scaled by this
ACT_SCALE = 1.0 / S_Z

_BUILD_CACHE = {}


def _emit_chains(nc, pools, terms, zp=None, final=True):
    """Paired A/B window matmuls for all four gates.

    terms: list of (kind, statA, statB, mov_fn); mov_fn(off) gives the moving
    AP for 512 gate columns at `off`. kind "fp8" = DoubleRow pair. A writes
    out partitions 0:64 (low half-gate), B writes 64:128 (high half-gate);
    both target the full [128,512] folded z tile.
    """
    psum = pools["psum"]
    new = zp is None
    if new:
        zp = {g: psum.tile([128, H], F32, name=f"z_{g}", tag="zp") for g in GORDER}
    nt = len(terms)
    for gate in GORDER:
        off = GATE_OFF[gate]
        z = zp[gate]
        for j, (kind, statA, statB, mov_fn) in enumerate(terms):
            start = new and j == 0
            stop = final and j == nt - 1
            pm = DR if kind == "fp8" else None
            nc.tensor.matmul(z[:, :], statA, mov_fn(off),
                             start=start, stop=False, perf_mode=pm,
                             skip_group_check=True)
            nc.tensor.matmul(z[:, :], statB, mov_fn(off + H),
                             start=False, stop=stop, perf_mode=pm,
                             skip_group_check=True)
    return zp


C0 = 128  # fast first chunk width; remaining 384 cols in one wide pass


def _emit_cell(nc, pools, zp, c_fold, first, idn, h8s_w, hT_w, bias_tiles=None):
    """Activations + state update. hT_w may be None (no dense consumer).

    Returns (tail_fn, h_fold). The c-update is split so chunk 0 (128 cols)
    clears the whole act->mul->transpose->copy chain first.
    """
    gates, temps = pools["gates"], pools["temps"]
    if bias_tiles is not None:
        for gate in GORDER:
            nc.vector.tensor_add(zp[gate][:, :], zp[gate][:, :], bias_tiles[gate])
    acts = {}
    for gate in ("f", "i", "g"):
        a = gates.tile([128, H], F32, tag="gact")
        nc.scalar.activation(a, zp[gate][:, :],
                             AF.Tanh if gate == "g" else AF.Sigmoid,
                             scale=ACT_SCALE)
        acts[gate] = a

    lo = slice(0, C0)
    hi = slice(C0, H)
    ig = temps.tile([128, H], F32, tag="tmp")
    nc.vector.tensor_mul(ig[:, lo], acts["i"][:, lo], acts["g"][:, lo])
    if first:
        nc.vector.tensor_copy(c_fold[:, lo], ig[:, lo])
    else:
        fc = temps.tile([128, H], F32, tag="tmp")
        nc.vector.tensor_mul(fc[:, lo], acts["f"][:, lo], c_fold[:, lo])
        nc.vector.tensor_add(c_fold[:, lo], fc[:, lo], ig[:, lo])
    nc.vector.tensor_mul(ig[:, hi], acts["i"][:, hi], acts["g"][:, hi])
    if first:
        nc.vector.tensor_copy(c_fold[:, hi], ig[:, hi])
    else:
        nc.vector.tensor_mul(fc[:, hi], acts["f"][:, hi], c_fold[:, hi])
        nc.vector.tensor_add(c_fold[:, hi], fc[:, hi], ig[:, hi])

    h_fold = pools["hfold"].tile([128, H], BF16, tag="hfold")

    def dump_chunk(hps, jj):
        blk = slice(128 * jj, 128 * (jj + 1))
        nc.tensor.transpose(hps[:, jj, :, :], h_fold[:, blk], idn[:, :])
        hview = hps[:, jj, :, :]
        nc.vector.tensor_scalar_mul(h8s_w[:, jj, :, 0:64], hview, S_H)
        nc.vector.tensor_scalar_mul(h8s_w[:, jj, :, 128:192], hview, S_H)
        if hT_w is not None:
            nc.vector.tensor_copy(hT_w[:, jj, :, :], hview)

    def tail(interleave_fn=None, dummy_src=None, ndum=0):
        hps = pools["psum_h"].tile([128, NP + 1, 2, 64], BF16, tag="hps")

        def dummies(n):
            # anti-HAM filler: keep the PE activity monitor fed through this
            # latency window; an idle gap halves the clock for ~3.4us.
            if dummy_src is not None:
                for _ in range(n):
                    nc.tensor.transpose(hps[:, NP, :, :], dummy_src[:, 0:128],
                                        idn[:, :])

        dummies(ndum // 2)
        # chunk 0 fast path: unblock the next cell's first DoubleRow pair
        tc0 = gates.tile([128, C0], F32, tag="tc0")
        nc.scalar.activation(tc0, c_fold[:, lo], AF.Tanh)
        o0 = gates.tile([128, C0], F32, tag="tc0")
        nc.scalar.activation(o0, zp["o"][:, lo], AF.Sigmoid, scale=ACT_SCALE)
        nc.vector.tensor_mul(h_fold[:, lo], o0, tc0)
        dump_chunk(hps, 0)
        if interleave_fn is not None:
            interleave_fn(0)
        dummies(ndum - ndum // 2)
        # chunks 1-3 in one wide pass
        tcr = gates.tile([128, H - C0], F32, tag="tcr")
        nc.scalar.activation(tcr, c_fold[:, hi], AF.Tanh)
        orr = gates.tile([128, H - C0], F32, tag="tcr")
        nc.scalar.activation(orr, zp["o"][:, hi], AF.Sigmoid, scale=ACT_SCALE)
        nc.vector.tensor_mul(h_fold[:, hi], orr, tcr)
        for jj in range(1, NP):
            dump_chunk(hps, jj)
            if interleave_fn is not None:
                interleave_fn(jj)

    return tail, h_fold


def _emit_pred_chain(nc, pools, wd_sb, hT_w):
    pd = pools["psum_p"].tile([128, BS], F32, tag="pd")

    def emit_chunk(jj):
        for i in range(2):
            nc.tensor.matmul(
                pd[:, :], wd_sb[:, jj, i, :], hT_w[:, jj, i, :],
                start=(jj == 0 and i == 0), stop=(jj == NP - 1 and i == 1),
            )
    return pd, emit_chunk


def _emit_pred_finish(nc, pools, pd, out_dram, step_idx, predT_w, bd_sb=None):
    pdv = pools["temps"].tile([128, BS], F32, tag="pdv")
    if bd_sb is not None:
        nc.vector.tensor_scalar_add(pdv, pd[:, :], bd_sb[:, 0:1])
    else:
        nc.vector.tensor_copy(pdv[:, :], pd[:, :])
    nc.sync.dma_start(out=out_dram[step_idx], in_=pdv[:, :])
    nc.vector.tensor_copy(predT_w[:, 0:64], pdv[:, :])
    nc.vector.tensor_copy(predT_w[:, 128:192], pdv[:, :])


def _build(has_b1, has_b2, has_bd, t_steps=T, out_steps=OUT_STEPS):
    nc = bacc.Bacc("TRN2", target_bir_lowering=False, debug=False)

    XT = nc.declare_dram_parameter("xt", [F, t_steps, 192], BF16, isOutput=False)
    W1S = nc.declare_dram_parameter("w1s", [F, G], BF16, isOutput=False)
    U18 = nc.declare_dram_parameter("u18", [128, NP, 2, G], F8, isOutput=False)
    W28 = nc.declare_dram_parameter("w28", [128, NP, 2, G], F8, isOutput=False)
    WD = nc.declare_dram_parameter("wd", [128, NP, 2, F], BF16, isOutput=False)
    IDN = nc.declare_dram_parameter("idn", [128, 128], BF16, isOutput=False)
    if has_b1:
        B1 = nc.declare_dram_parameter("b1f", [4, 128, H], F32, isOutput=False)
    if has_b2:
        B2 = nc.declare_dram_parameter("b2f", [4, 128, H], F32, isOutput=False)
    if has_bd:
        BD = nc.declare_dram_parameter("bdf", [128, 1], F32, isOutput=False)
    OUT = nc.declare_dram_parameter("out", [out_steps, F, BS], F32, isOutput=True)

    with tile.TileContext(nc) as tc, ExitStack() as ctx:
        consts = ctx.enter_context(tc.tile_pool(name="consts", bufs=1))
        pools = {
            "psum": ctx.enter_context(tc.tile_pool(name="psum", bufs=6, space="PSUM")),
            "psum_h": ctx.enter_context(tc.tile_pool(name="psum_h", bufs=1, space="PSUM")),
            "psum_p": ctx.enter_context(tc.tile_pool(name="psum_p", bufs=1, space="PSUM")),
            "gates": ctx.enter_context(tc.tile_pool(name="gates", bufs=8)),
            "temps": ctx.enter_context(tc.tile_pool(name="temps", bufs=4)),
            "hfold": ctx.enter_context(tc.tile_pool(name="hfold", bufs=2)),
        }

        xt_sb = consts.tile([F, t_steps, 192], BF16)
        w1s_sb = consts.tile([F, G], BF16)
        u18_sb = consts.tile([128, NP, 2, G], F8)
        w28_sb = consts.tile([128, NP, 2, G], F8)
        wd_sb = consts.tile([128, NP, 2, F], BF16)
        idn_sb = consts.tile([128, 128], BF16)
        nc.sync.dma_start(out=xt_sb[:], in_=XT[:])
        nc.sync.dma_start(out=w1s_sb[:], in_=W1S[:])
        for j in range(NP):  # split big weight DMAs so early steps start sooner
            nc.sync.dma_start(out=u18_sb[:, j, :, :], in_=U18[:, j, :, :])
        for j in range(NP):
            nc.sync.dma_start(out=w28_sb[:, j, :, :], in_=W28[:, j, :, :])
        nc.sync.dma_start(out=wd_sb[:], in_=WD[:])
        nc.sync.dma_start(out=idn_sb[:], in_=IDN[:])

        b1_tiles = b2_tiles = None
        if has_b1:
            b1_sb = consts.tile([4, 128, H], F32)
            nc.sync.dma_start(out=b1_sb[:], in_=B1[:])
            b1_tiles = {g: b1_sb[i] for i, g in enumerate(("i", "f", "g", "o"))}
        if has_b2:
            b2_sb = consts.tile([4, 128, H], F32)
            nc.sync.dma_start(out=b2_sb[:], in_=B2[:])
            b2_tiles = {g: b2_sb[i] for i, g in enumerate(("i", "f", "g", "o"))}
        bd_sb = None
        if has_bd:
            bd_sb = consts.tile([128, 1], F32)
            nc.sync.dma_start(out=bd_sb[:], in_=BD[:])

        c_fold = consts.tile([128, H], F32)  # persistent cell state

        # double-buffered stationaries; padding columns zeroed once
        h8s_bufs = [consts.tile([128, NP, 2, 192], F8, name=f"h8s{i}") for i in range(2)]
        hT_bufs = [consts.tile([128, NP, 2, BS], BF16, name=f"hT{i}") for i in range(2)]
        predT_bufs = [consts.tile([128, 192], BF16, name=f"pT{i}") for i in range(2)]
        for b in h8s_bufs:
            nc.vector.memset(b[:], 0.0)
        for b in predT_bufs:
            nc.vector.memset(b[:], 0.0)

        def fp8_term(h8s_r, j, w_sb):
            return ("fp8", h8s_r[:, j, :, 0:128], h8s_r[:, j, :, 64:192],
                    lambda off, j=j: w_sb[:, j, :, off:off + H])

        def x_term(t):
            return ("bf16", xt_sb[:, t, 0:128], xt_sb[:, t, 64:192],
                    lambda off: w1s_sb[:, off:off + H])

        def pred_term(predT_r):
            return ("bf16", predT_r[:, 0:128], predT_r[:, 64:192],
                    lambda off: w1s_sb[:, off:off + H])

        # ---- warmup ----
        cell = 0
        tail = None
        prev_h = None
        for t in range(t_steps):
            h8s_w = h8s_bufs[cell % 2]
            hT_w = hT_bufs[0] if t == t_steps - 1 else None
            if t == 0:
                zp = _emit_chains(nc, pools, [x_term(0)], final=True)
                tail, prev_h = _emit_cell(nc, pools, zp, c_fold, True, idn_sb,
                                          h8s_w, hT_w, b1_tiles)
                cell += 1
                continue
            # pre-start this cell's x chains: they cover the previous tail's
            # latency window on the PE
            zp = _emit_chains(nc, pools, [x_term(t)], final=False)
            tail()
            h8s_r = h8s_bufs[(cell - 1) % 2]
            terms = [fp8_term(h8s_r, j, u18_sb) for j in range(NP)]
            zp = _emit_chains(nc, pools, terms, zp=zp, final=True)
            tail, prev_h = _emit_cell(nc, pools, zp, c_fold, False, idn_sb,
                                      h8s_w, hT_w, b1_tiles)
            cell += 1
        # ---- pred0 interleaved into the last warmup tail ----
        predT_w = predT_bufs[0]
        pd, pred_chunk = _emit_pred_chain(nc, pools, wd_sb, hT_bufs[0])
        tail(pred_chunk)
        _emit_pred_finish(nc, pools, pd, OUT, 0, predT_w, bd_sb)

        # ---- autoregressive ----
        for t in range(out_steps - 1):
            predT_r = predT_bufs[t % 2]
            predT_w = predT_bufs[(t + 1) % 2]
            h8s_r = h8s_bufs[(cell - 1) % 2]
            h8s_w = h8s_bufs[cell % 2]
            terms1 = [fp8_term(h8s_r, j, u18_sb) for j in range(NP)] \
                + [pred_term(predT_r)]
            zp = _emit_chains(nc, pools, terms1, final=True)
            tail1, h1_fold = _emit_cell(nc, pools, zp, c_fold, False, idn_sb,
                                        h8s_w, None, b1_tiles)
            tail1(None, dummy_src=prev_h, ndum=10)
            cell += 1
            h8s_r = h8s_bufs[(cell - 1) % 2]
            h8s_w = h8s_bufs[cell % 2]
            hT_w = hT_bufs[cell % 2]
            terms2 = [fp8_term(h8s_r, j, w28_sb) for j in range(NP)]
            zp = _emit_chains(nc, pools, terms2, final=True)
            tail2, prev_h = _emit_cell(nc, pools, zp, c_fold, False, idn_sb,
                                       h8s_w, hT_w, b2_tiles)
            pd, pred_chunk = _emit_pred_chain(nc, pools, wd_sb, hT_w)
            tail2(pred_chunk, dummy_src=h1_fold, ndum=8)
            _emit_pred_finish(nc, pools, pd, OUT, t + 1, predT_w, bd_sb)
            cell += 1

    nc.compile()
    return nc


def _fold_bias(b, scale):
    out = np.zeros((4, 128, H), np.float32)
    for gi, gname in enumerate(("i", "f", "g", "o")):
        off = GATE_OFF[gname]
        out[gi, 0:64, :] = b[off:off + H][None, :] * scale
        out[gi, 64:128, :] = b[off + H:off + 2 * H][None, :] * scale
    return out


def _prep_fp8_pairs(Wmat, scale):
    """[1024, 4096] -> [128, NP, 2, G] fp8 with plane pairs (j, j+4)."""
    f8 = ml_dtypes.float8_e4m3
    kt = Wmat.reshape(8, 128, G)
    out = np.empty((128, NP, 2, G), np.float32)
    for j in range(NP):
        out[:, j, 0, :] = kt[j]
        out[:, j, 1, :] = kt[j + 4]
    return np.clip(out * scale, -240.0, 240.0).astype(f8)


def kernel(inputs, mean, var, W1, U1, b1, W2, U2, b2, Wd, bd):
    x = np.asarray(inputs, np.float32)
    mean = np.asarray(mean, np.float32)
    var = np.asarray(var, np.float32)
    inv = 1.0 / np.sqrt(var + EPS)
    xn = ((x - mean) * inv - mean) * inv  # reference normalizes twice

    bf = ml_dtypes.bfloat16
    W1 = np.asarray(W1, np.float32)
    U1 = np.asarray(U1, np.float32)
    W2U2 = np.asarray(W2, np.float32) + np.asarray(U2, np.float32)
    Wd = np.asarray(Wd, np.float32)

    wdk = Wd.reshape(8, 128, F)
    wd_sb = np.empty((128, NP, 2, F), np.float32)
    for j in range(NP):
        wd_sb[:, j, 0, :] = wdk[j]
        wd_sb[:, j, 1, :] = wdk[j + 4]

    b1 = np.asarray(b1, np.float32)
    b2 = np.asarray(b2, np.float32)
    bd = np.asarray(bd, np.float32)
    has_b1 = bool(np.any(b1))
    has_b2 = bool(np.any(b2))
    has_bd = bool(np.any(bd))

    key = (has_b1, has_b2, has_bd)
    if key not in _BUILD_CACHE:
        _BUILD_CACHE[key] = _build(*key)
    nc = _BUILD_CACHE[key]

    shared = {
        "w1s": (W1 * S_Z).astype(bf),
        "u18": _prep_fp8_pairs(U1, S_W),
        "w28": _prep_fp8_pairs(W2U2, S_W),
        "wd": wd_sb.astype(bf),
        "idn": np.eye(128, dtype=np.float32).astype(bf),
    }
    if has_b1:
        shared["b1f"] = _fold_bias(b1, S_Z)
    if has_b2:
        shared["b2f"] = _fold_bias(b2, S_Z)
    if has_bd:
        shared["bdf"] = bd.reshape(128, 1).astype(np.float32)

    in_maps = []
    for c in range(NCORES):
        shard = xn[c * BS:(c + 1) * BS]              # [64, 64, 128]
        xtc = np.ascontiguousarray(shard.transpose(2, 1, 0))  # [F, T, BS]
        xt = np.zeros((F, T, 192), np.float32)
        xt[:, :, 0:64] = xtc
        xt[:, :, 128:192] = xtc
        m = dict(shared)
        m["xt"] = xt.astype(bf)
        in_maps.append(m)

    res = run_bass_kernel_spmd(nc, in_maps, core_ids=list(range(NCORES)))
    kernel.last_results = res

    parts = [res.results[c]["out"].transpose(2, 0, 1) for c in range(NCORES)]
    return np.ascontiguousarray(np.concatenate(parts, axis=0), dtype=np.float32)


# revision 4
# speedup vs baseline: 1.3013x; 1.3013x over previous
"""Trainium2 Bass kernel for the AutoRegressiveLSTM problem — v4.

Data-parallel over batch (512 -> 64 rows/core, 8 cores). Design:

- Gate matmuls are fp8-e4m3 DoubleRow pairs (two 128-unit k-tiles per
  instruction) in FULL-ARRAY tile mode (128,128); every PE instruction
  (gates, transposes, x/pred terms, dense) shares ONE tile mode -> no
  TensorE mode-switch drains. ~39 TMAC/s measured on the gate stream.
- Folded z layout (swapped): PSUM partitions 64:128 hold gate cols
  [off, off+512), partitions 0:64 hold [off+512, off+1024). With this fold
  the fp8 stationary tile needs h written at ONE place (cols 64:128 of a
  192-wide tile): window [0:128] reads (0|h) -> partitions 64:128, window
  [64:192] reads (h|0) -> partitions 0:64. One fp8 copy per chunk instead
  of two; same trick for the x / pred bf16 stationaries.
- Full-width transposes (4/cell) produce (k+4, k) plane pairs directly.
- fp8 scales: h*128, U-weights*256 -> z scaled by 32768 in PSUM;
  activation(scale=1/32768) undoes it exactly.
- Tail is pipelined in 3 chunk-groups (128/128/256 cols): each group runs
  sigmoid(o)/tanh(c)/h-mul/transpose/fp8-copy for its columns so the next
  cell's DoubleRow pair j starts as soon as chunk j lands. Critical fp8
  copies run on the Scalar engine (activation Copy w/ scale) to keep the
  Vector queue clear for the c-update chain. Gate order (i,g,f,o) makes
  i*g ready at 50% of the stream and f*c at 75%.
- Anti-HAM dummy transposes fill every tail's latency window: an idle PE
  gap makes the activity monitor halve the clock for ~3.4us afterwards
  (measured), so idle windows cost double.

Max-rel error vs fp32 reference ~1.1e-2 on hardware (tolerance 2e-2).
"""

from contextlib import ExitStack

import numpy as np
import ml_dtypes

import concourse.bass as bass
import concourse.tile as tile
from concourse import bacc, mybir
from concourse.bass_utils import run_bass_kernel_spmd

BF16 = mybir.dt.bfloat16
F8 = mybir.dt.float8e4
F32 = mybir.dt.float32
AF = mybir.ActivationFunctionType
DR = mybir.MatmulPerfMode.DoubleRow

NCORES = 8
B_FULL = 512
BS = B_FULL // NCORES   # 64 batch rows per core
T = 64                  # warmup sequence length
F = 128                 # features
U = 1024                # LSTM units
G = 4 * U               # 4096 gate columns
NP = 4                  # k-tile pairs; transpose chunk j yields planes (j+4, j)
OUT_STEPS = 32
EPS = 1e-7
H = 512                 # half-gate width
GATE_OFF = {"i": 0, "f": U, "g": 2 * U, "o": 3 * U}
GORDER = ("i", "g", "f", "o")  # i*g ready at 50%, f*c at 75%, o last

S_H = 128.0             # h quantization scale
S_W = 256.0             # U1 / (W2+U2) quantization scale
S_Z = S_H * S_W         # 32768; z lands in PSUM scaled by this
ACT_SCALE = 1.0 / S_Z

# tail chunk groups: (col range, transpose chunks)
TGROUPS = [(slice(0, 128), (0,)), (slice(128, 256), (1,)), (slice(256, 512), (2, 3))]

_BUILD_CACHE = {}


def _emit_chains(nc, pools, terms, zp=None, final=True):
    """Paired A/B window matmuls for all four gates.

    terms: list of (kind, statA, statB, mov_fn); mov_fn(off) gives the moving
    AP for 512 gate columns at `off`. kind "fp8" = DoubleRow pair. With the
    swapped fold, window A (0|h) writes partitions 64:128 = cols off..off+512
    and window B (h|0) writes partitions 0:64 = cols off+512..off+1024.
    """
    psum = pools["psum"]
    new = zp is None
    if new:
        zp = {g: psum.tile([128, H], F32, name=f"z_{g}", tag="zp") for g in GORDER}
    nt = len(terms)
    for gate in GORDER:
        off = GATE_OFF[gate]
        z = zp[gate]
        for j, (kind, statA, statB, mov_fn) in enumerate(terms):
            start = new and j == 0
            stop = final and j == nt - 1
            pm = DR if kind == "fp8" else None
            nc.tensor.matmul(z[:, :], statA, mov_fn(off),
                             start=start, stop=False, perf_mode=pm,
                             skip_group_check=True)
            nc.tensor.matmul(z[:, :], statB, mov_fn(off + H),
                             start=False, stop=stop, perf_mode=pm,
                             skip_group_check=True)
    return zp


def _emit_cell(nc, pools, zp, c_fold, first, idn, h8s_w, hT_w, bias_tiles=None):
    """Activations + state update. hT_w may be None (no dense consumer).

    Returns (tail_fn, h_fold). The c-update and tail run in 3 chunk groups
    so the first 128-col chunk clears the act->mul->transpose->copy chain
    before the wide work.
    """
    gates, temps = pools["gates"], pools["temps"]
    if bias_tiles is not None:
        for gate in GORDER:
            nc.vector.tensor_add(zp[gate][:, :], zp[gate][:, :], bias_tiles[gate])
    acts = {}
    for gate in ("i", "g", "f"):
        a = gates.tile([128, H], F32, tag="gact")
        nc.scalar.activation(a, zp[gate][:, :],
                             AF.Tanh if gate == "g" else AF.Sigmoid,
                             scale=ACT_SCALE)
        acts[gate] = a

    ig = temps.tile([128, H], F32, tag="tmp")
    fc = None if first else temps.tile([128, H], F32, tag="tmp")

    def update_c(sl):
        nc.vector.tensor_mul(ig[:, sl], acts["i"][:, sl], acts["g"][:, sl])
        if first:
            nc.vector.tensor_copy(c_fold[:, sl], ig[:, sl])
        else:
            nc.vector.tensor_mul(fc[:, sl], acts["f"][:, sl], c_fold[:, sl])
            nc.vector.tensor_add(c_fold[:, sl], fc[:, sl], ig[:, sl])

    h_fold = pools["hfold"].tile([128, H], BF16, tag="hfold")

    def tail(interleave_fn=None, dummy_src=None, ndum=(0, 0, 0)):
        hps = pools["psum_h"].tile([128, NP + 1, 2, 64], BF16, tag="hps")

        def dummies(n):
            # anti-HAM filler: keep the PE activity monitor fed through this
            # latency window; an idle gap halves the clock for ~3.4us.
            if dummy_src is not None:
                for _ in range(n):
                    nc.tensor.transpose(hps[:, NP, :, :], dummy_src[:, 0:128],
                                        idn[:, :])

        for gi, (sl, chunks) in enumerate(TGROUPS):
            dummies(ndum[gi])
            update_c(sl)
            w = sl.stop - sl.start
            tc_t = gates.tile([128, w], F32, tag=f"tc{gi}")
            nc.scalar.activation(tc_t, c_fold[:, sl], AF.Tanh)
            o_t = gates.tile([128, w], F32, tag=f"tc{gi}")
            nc.scalar.activation(o_t, zp["o"][:, sl], AF.Sigmoid, scale=ACT_SCALE)
            nc.vector.tensor_mul(h_fold[:, sl], o_t, tc_t)
            for ci, jj in enumerate(chunks):
                blk = slice(128 * jj, 128 * (jj + 1))
                nc.tensor.transpose(hps[:, jj, :, :], h_fold[:, blk], idn[:, :])
                hview = hps[:, jj, :, :]
                if ci == 0:
                    # critical handoff copy on the Scalar engine
                    nc.scalar.activation(h8s_w[:, jj, :, 64:128], hview,
                                         AF.Copy, scale=S_H)
                else:
                    nc.vector.tensor_scalar_mul(h8s_w[:, jj, :, 64:128], hview, S_H)
                if hT_w is not None:
                    nc.vector.tensor_copy(hT_w[:, jj, :, :], hview)
                if interleave_fn is not None:
                    interleave_fn(jj)

    return tail, h_fold


def _emit_pred_chain(nc, pools, wd_sb, hT_w):
    pd = pools["psum_p"].tile([128, BS], F32, tag="pd")

    def emit_chunk(jj):
        for i in range(2):
            nc.tensor.matmul(
                pd[:, :], wd_sb[:, jj, i, :], hT_w[:, jj, i, :],
                start=(jj == 0 and i == 0), stop=(jj == NP - 1 and i == 1),
            )
    return pd, emit_chunk


def _emit_pred_finish(nc, pools, pd, out_dram, step_idx, predT_w, bd_sb=None):
    pdv = pools["temps"].tile([128, BS], F32, tag="pdv")
    if bd_sb is not None:
        nc.vector.tensor_scalar_add(pdv, pd[:, :], bd_sb[:, 0:1])
    else:
        nc.vector.tensor_copy(pdv[:, :], pd[:, :])
    nc.sync.dma_start(out=out_dram[step_idx], in_=pdv[:, :])
    nc.vector.tensor_copy(predT_w[:, 64:128], pdv[:, :])


def _build(has_b1, has_b2, has_bd, t_steps=T, out_steps=OUT_STEPS):
    nc = bacc.Bacc("TRN2", target_bir_lowering=False, debug=False)

    XT = nc.declare_dram_parameter("xt", [F, t_steps, 192], BF16, isOutput=False)
    W1S = nc.declare_dram_parameter("w1s", [F, G], BF16, isOutput=False)
    U18 = nc.declare_dram_parameter("u18", [128, NP, 2, G], F8, isOutput=False)
    W28 = nc.declare_dram_parameter("w28", [128, NP, 2, G], F8, isOutput=False)
    WD = nc.declare_dram_parameter("wd", [128, NP, 2, F], BF16, isOutput=False)
    IDN = nc.declare_dram_parameter("idn", [128, 128], BF16, isOutput=False)
    if has_b1:
        B1 = nc.declare_dram_parameter("b1f", [4, 128, H], F32, isOutput=False)
    if has_b2:
        B2 = nc.declare_dram_parameter("b2f", [4, 128, H], F32, isOutput=False)
    if has_bd:
        BD = nc.declare_dram_parameter("bdf", [128, 1], F32, isOutput=False)
    OUT = nc.declare_dram_parameter("out", [out_steps, F, BS], F32, isOutput=True)

    with tile.TileContext(nc) as tc, ExitStack() as ctx:
        consts = ctx.enter_context(tc.tile_pool(name="consts", bufs=1))
        pools = {
            "psum": ctx.enter_context(tc.tile_pool(name="psum", bufs=6, space="PSUM")),
            "psum_h": ctx.enter_context(tc.tile_pool(name="psum_h", bufs=1, space="PSUM")),
            "psum_p": ctx.enter_context(tc.tile_pool(name="psum_p", bufs=1, space="PSUM")),
            "gates": ctx.enter_context(tc.tile_pool(name="gates", bufs=8)),
            "temps": ctx.enter_context(tc.tile_pool(name="temps", bufs=4)),
            "hfold": ctx.enter_context(tc.tile_pool(name="hfold", bufs=2)),
        }

        xt_sb = consts.tile([F, t_steps, 192], BF16)
        w1s_sb = consts.tile([F, G], BF16)
        u18_sb = consts.tile([128, NP, 2, G], F8)
        w28_sb = consts.tile([128, NP, 2, G], F8)
        wd_sb = consts.tile([128, NP, 2, F], BF16)
        idn_sb = consts.tile([128, 128], BF16)
        nc.sync.dma_start(out=xt_sb[:], in_=XT[:])
        nc.sync.dma_start(out=w1s_sb[:], in_=W1S[:])
        for j in range(NP):  # split big weight DMAs so early steps start sooner
            nc.sync.dma_start(out=u18_sb[:, j, :, :], in_=U18[:, j, :, :])
        for j in range(NP):
            nc.sync.dma_start(out=w28_sb[:, j, :, :], in_=W28[:, j, :, :])
        nc.sync.dma_start(out=wd_sb[:], in_=WD[:])
        nc.sync.dma_start(out=idn_sb[:], in_=IDN[:])

        b1_tiles = b2_tiles = None
        if has_b1:
            b1_sb = consts.tile([4, 128, H], F32)
            nc.sync.dma_start(out=b1_sb[:], in_=B1[:])
            b1_tiles = {g: b1_sb[i] for i, g in enumerate(("i", "f", "g", "o"))}
        if has_b2:
            b2_sb = consts.tile([4, 128, H], F32)
            nc.sync.dma_start(out=b2_sb[:], in_=B2[:])
            b2_tiles = {g: b2_sb[i] for i, g in enumerate(("i", "f", "g", "o"))}
        bd_sb = None
        if has_bd:
            bd_sb = consts.tile([128, 1], F32)
            nc.sync.dma_start(out=bd_sb[:], in_=BD[:])

        c_fold = consts.tile([128, H], F32)  # persistent cell state

        # double-buffered stationaries; padding columns zeroed once
        h8s_bufs = [consts.tile([128, NP, 2, 192], F8, name=f"h8s{i}") for i in range(2)]
        hT_bufs = [consts.tile([128, NP, 2, BS], BF16, name=f"hT{i}") for i in range(2)]
        predT_bufs = [consts.tile([128, 192], BF16, name=f"pT{i}") for i in range(2)]
        for b in h8s_bufs:
            nc.vector.memset(b[:], 0.0)
        for b in predT_bufs:
            nc.vector.memset(b[:], 0.0)

        def fp8_term(h8s_r, j, w_sb):
            return ("fp8", h8s_r[:, j, :, 0:128], h8s_r[:, j, :, 64:192],
                    lambda off, j=j: w_sb[:, j, :, off:off + H])

        def x_term(t):
            return ("bf16", xt_sb[:, t, 0:128], xt_sb[:, t, 64:192],
                    lambda off: w1s_sb[:, off:off + H])

        def pred_term(predT_r):
            return ("bf16", predT_r[:, 0:128], predT_r[:, 64:192],
                    lambda off: w1s_sb[:, off:off + H])

        WARM_DUM = (3, 2, 2)
        AR_DUM = (6, 4, 4)

        # ---- warmup ----
        cell = 0
        tail = None
        prev_h = None
        for t in range(t_steps):
            h8s_w = h8s_bufs[cell % 2]
            hT_w = hT_bufs[0] if t == t_steps - 1 else None
            if t == 0:
                zp = _emit_chains(nc, pools, [x_term(0)], final=True)
                tail, prev_h = _emit_cell(nc, pools, zp, c_fold, True, idn_sb,
                                          h8s_w, hT_w, b1_tiles)
                cell += 1
                continue
            # pre-start this cell's x chains: they cover the previous tail's
            # latency window on the PE
            zp = _emit_chains(nc, pools, [x_term(t)], final=False)
            tail(None, dummy_src=prev_h, ndum=WARM_DUM)
            h8s_r = h8s_bufs[(cell - 1) % 2]
            terms = [fp8_term(h8s_r, j, u18_sb) for j in range(NP)]
            zp = _emit_chains(nc, pools, terms, zp=zp, final=True)
            tail, prev_h = _emit_cell(nc, pools, zp, c_fold, False, idn_sb,
                                      h8s_w, hT_w, b1_tiles)
            cell += 1
        # ---- pred0 interleaved into the last warmup tail ----
        predT_w = predT_bufs[0]
        pd, pred_chunk = _emit_pred_chain(nc, pools, wd_sb, hT_bufs[0])
        tail(pred_chunk, dummy_src=prev_h, ndum=WARM_DUM)
        _emit_pred_finish(nc, pools, pd, OUT, 0, predT_w, bd_sb)

        # ---- autoregressive ----
        for t in range(out_steps - 1):
            predT_r = predT_bufs[t % 2]
            predT_w = predT_bufs[(t + 1) % 2]
            h8s_r = h8s_bufs[(cell - 1) % 2]
            h8s_w = h8s_bufs[cell % 2]
            terms1 = [fp8_term(h8s_r, j, u18_sb) for j in range(NP)] \
                + [pred_term(predT_r)]
            zp = _emit_chains(nc, pools, terms1, final=True)
            tail1, h1_fold = _emit_cell(nc, pools, zp, c_fold, False, idn_sb,
                                        h8s_w, None, b1_tiles)
            tail1(None, dummy_src=prev_h, ndum=AR_DUM)
            cell += 1
            h8s_r = h8s_bufs[(cell - 1) % 2]
            h8s_w = h8s_bufs[cell % 2]
            hT_w = hT_bufs[cell % 2]
            terms2 = [fp8_term(h8s_r, j, w28_sb) for j in range(NP)]
            zp = _emit_chains(nc, pools, terms2, final=True)
            tail2, prev_h = _emit_cell(nc, pools, zp, c_fold, False, idn_sb,
                                       h8s_w, hT_w, b2_tiles)
            pd, pred_chunk = _emit_pred_chain(nc, pools, wd_sb, hT_w)
            tail2(pred_chunk, dummy_src=h1_fold, ndum=AR_DUM)
            _emit_pred_finish(nc, pools, pd, OUT, t + 1, predT_w, bd_sb)
            cell += 1

    nc.compile()
    return nc


def _fold_bias(b, scale):
    # swapped fold: partitions 64:128 = low half-gate, 0:64 = high half-gate
    out = np.zeros((4, 128, H), np.float32)
    for gi, gname in enumerate(("i", "f", "g", "o")):
        off = GATE_OFF[gname]
        out[gi, 64:128, :] = b[off:off + H][None, :] * scale
        out[gi, 0:64, :] = b[off + H:off + 2 * H][None, :] * scale
    return out


def _prep_fp8_pairs(Wmat, scale):
    """[1024, 4096] -> [128, NP, 2, G] fp8 with plane pairs (j+4, j)."""
    f8 = ml_dtypes.float8_e4m3
    kt = Wmat.reshape(8, 128, G)
    out = np.empty((128, NP, 2, G), np.float32)
    for j in range(NP):
        out[:, j, 0, :] = kt[j + 4]
        out[:, j, 1, :] = kt[j]
    return np.clip(out * scale, -240.0, 240.0).astype(f8)


def kernel(inputs, mean, var, W1, U1, b1, W2, U2, b2, Wd, bd):
    x = np.asarray(inputs, np.float32)
    mean = np.asarray(mean, np.float32)
    var = np.asarray(var, np.float32)
    inv = 1.0 / np.sqrt(var + EPS)
    xn = ((x - mean) * inv - mean) * inv  # reference normalizes twice

    bf = ml_dtypes.bfloat16
    W1 = np.asarray(W1, np.float32)
    U1 = np.asarray(U1, np.float32)
    W2U2 = np.asarray(W2, np.float32) + np.asarray(U2, np.float32)
    Wd = np.asarray(Wd, np.float32)

    wdk = Wd.reshape(8, 128, F)
    wd_sb = np.empty((128, NP, 2, F), np.float32)
    for j in range(NP):
        wd_sb[:, j, 0, :] = wdk[j + 4]
        wd_sb[:, j, 1, :] = wdk[j]

    b1 = np.asarray(b1, np.float32)
    b2 = np.asarray(b2, np.float32)
    bd = np.asarray(bd, np.float32)
    has_b1 = bool(np.any(b1))
    has_b2 = bool(np.any(b2))
    has_bd = bool(np.any(bd))

    key = (has_b1, has_b2, has_bd)
    if key not in _BUILD_CACHE:
        _BUILD_CACHE[key] = _build(*key)
    nc = _BUILD_CACHE[key]

    shared = {
        "w1s": (W1 * S_Z).astype(bf),
        "u18": _prep_fp8_pairs(U1, S_W),
        "w28": _prep_fp8_pairs(W2U2, S_W),
        "wd": wd_sb.astype(bf),
        "idn": np.eye(128, dtype=np.float32).astype(bf),
    }
    if has_b1:
        shared["b1f"] = _fold_bias(b1, S_Z)
    if has_b2:
        shared["b2f"] = _fold_bias(b2, S_Z)
    if has_bd:
        shared["bdf"] = bd.reshape(128, 1).astype(np.float32)

    in_maps = []
    for c in range(NCORES):
        shard = xn[c * BS:(c + 1) * BS]              # [64, 64, 128]
        xtc = np.ascontiguousarray(shard.transpose(2, 1, 0))  # [F, T, BS]
        xt = np.zeros((F, T, 192), np.float32)
        xt[:, :, 64:128] = xtc
        m = dict(shared)
        m["xt"] = xt.astype(bf)
        in_maps.append(m)

    res = run_bass_kernel_spmd(nc, in_maps, core_ids=list(range(NCORES)))
    kernel.last_results = res

    parts = [res.results[c]["out"].transpose(2, 0, 1) for c in range(NCORES)]
    return np.ascontiguousarray(np.concatenate(parts, axis=0), dtype=np.float32)


# revision 5
# speedup vs baseline: 1.3142x; 1.0099x over previous
"""Trainium2 Bass kernel for the AutoRegressiveLSTM problem — v4.

Data-parallel over batch (512 -> 64 rows/core, 8 cores). Design:

- Gate matmuls are fp8-e4m3 DoubleRow pairs (two 128-unit k-tiles per
  instruction) in FULL-ARRAY tile mode (128,128); every PE instruction
  (gates, transposes, x/pred terms, dense) shares ONE tile mode -> no
  TensorE mode-switch drains. ~39 TMAC/s measured on the gate stream.
- Folded z layout (swapped): PSUM partitions 64:128 hold gate cols
  [off, off+512), partitions 0:64 hold [off+512, off+1024). With this fold
  the fp8 stationary tile needs h written at ONE place (cols 64:128 of a
  192-wide tile): window [0:128] reads (0|h) -> partitions 64:128, window
  [64:192] reads (h|0) -> partitions 0:64. One fp8 copy per chunk instead
  of two; same trick for the x / pred bf16 stationaries.
- Full-width transposes (4/cell) produce (k+4, k) plane pairs directly.
- fp8 scales: h*128, U-weights*256 -> z scaled by 32768 in PSUM;
  activation(scale=1/32768) undoes it exactly.
- Tail is pipelined in 3 chunk-groups (128/128/256 cols): each group runs
  sigmoid(o)/tanh(c)/h-mul/transpose/fp8-copy for its columns so the next
  cell's DoubleRow pair j starts as soon as chunk j lands. Critical fp8
  copies run on the Scalar engine (activation Copy w/ scale) to keep the
  Vector queue clear for the c-update chain. Gate order (i,g,f,o) makes
  i*g ready at 50% of the stream and f*c at 75%.
- Anti-HAM dummy transposes fill every tail's latency window: an idle PE
  gap makes the activity monitor halve the clock for ~3.4us afterwards
  (measured), so idle windows cost double.

Max-rel error vs fp32 reference ~1.1e-2 on hardware (tolerance 2e-2).
"""

from contextlib import ExitStack

import numpy as np
import ml_dtypes

import concourse.bass as bass
import concourse.tile as tile
from concourse import bacc, mybir
from concourse.bass_utils import run_bass_kernel_spmd

BF16 = mybir.dt.bfloat16
F8 = mybir.dt.float8e4
F32 = mybir.dt.float32
AF = mybir.ActivationFunctionType
DR = mybir.MatmulPerfMode.DoubleRow

NCORES = 8
B_FULL = 512
BS = B_FULL // NCORES   # 64 batch rows per core
T = 64                  # warmup sequence length
F = 128                 # features
U = 1024                # LSTM units
G = 4 * U               # 4096 gate columns
NP = 4                  # k-tile pairs; transpose chunk j yields planes (j+4, j)
OUT_STEPS = 32
EPS = 1e-7
H = 512                 # half-gate width
GATE_OFF = {"i": 0, "f": U, "g": 2 * U, "o": 3 * U}
GORDER = ("i", "g", "f", "o")  # i*g ready at 50%, f*c at 75%, o last

S_H = 128.0             # h quantization scale
S_W = 256.0             # U1 / (W2+U2) quantization scale
S_Z = S_H * S_W         # 32768; z lands in PSUM scaled by this
ACT_SCALE = 1.0 / S_Z

# tail chunk groups: (col range, transpose chunks)
TGROUPS = [(slice(0, 128), (0,)), (slice(128, 256), (1,)), (slice(256, 512), (2, 3))]

_BUILD_CACHE = {}


def _emit_chains(nc, pools, terms, zp=None, final=True):
    """Paired A/B window matmuls for all four gates.

    terms: list of (kind, statA, statB, mov_fn); mov_fn(off) gives the moving
    AP for 512 gate columns at `off`. kind "fp8" = DoubleRow pair. With the
    swapped fold, window A (0|h) writes partitions 64:128 = cols off..off+512
    and window B (h|0) writes partitions 0:64 = cols off+512..off+1024.
    """
    psum = pools["psum"]
    new = zp is None
    if new:
        zp = {g: psum.tile([128, H], F32, name=f"z_{g}", tag="zp") for g in GORDER}
    nt = len(terms)
    for gate in GORDER:
        off = GATE_OFF[gate]
        z = zp[gate]
        for j, (kind, statA, statB, mov_fn) in enumerate(terms):
            start = new and j == 0
            stop = final and j == nt - 1
            pm = DR if kind == "fp8" else None
            nc.tensor.matmul(z[:, :], statA, mov_fn(off),
                             start=start, stop=False, perf_mode=pm,
                             skip_group_check=True)
            nc.tensor.matmul(z[:, :], statB, mov_fn(off + H),
                             start=False, stop=stop, perf_mode=pm,
                             skip_group_check=True)
    return zp


def _emit_cell(nc, pools, zp, c_fold, first, idn, h8s_w, hT_w, bias_tiles=None):
    """Activations + state update. hT_w may be None (no dense consumer).

    Returns (tail_fn, h_fold). The c-update and tail run in 3 chunk groups
    so the first 128-col chunk clears the act->mul->transpose->copy chain
    before the wide work.
    """
    gates, temps = pools["gates"], pools["temps"]
    if bias_tiles is not None:
        for gate in GORDER:
            nc.vector.tensor_add(zp[gate][:, :], zp[gate][:, :], bias_tiles[gate])
    acts = {}
    for gate in ("i", "g", "f"):
        a = gates.tile([128, H], F32, tag="gact")
        nc.scalar.activation(a, zp[gate][:, :],
                             AF.Tanh if gate == "g" else AF.Sigmoid,
                             scale=ACT_SCALE)
        acts[gate] = a

    ig = temps.tile([128, H], F32, tag="tmp")
    fc = None if first else temps.tile([128, H], F32, tag="tmp")

    def update_c(sl):
        nc.vector.tensor_mul(ig[:, sl], acts["i"][:, sl], acts["g"][:, sl])
        if first:
            nc.vector.tensor_copy(c_fold[:, sl], ig[:, sl])
        else:
            nc.vector.tensor_mul(fc[:, sl], acts["f"][:, sl], c_fold[:, sl])
            nc.vector.tensor_add(c_fold[:, sl], fc[:, sl], ig[:, sl])

    h_fold = pools["hfold"].tile([128, H], BF16, tag="hfold")

    def tail(interleave_fn=None, dummy_src=None, ndum=(0, 0, 0)):
        hps = pools["psum_h"].tile([128, NP + 1, 2, 64], BF16, tag="hps")

        def dummies(n):
            # anti-HAM filler: keep the PE activity monitor fed through this
            # latency window; an idle gap halves the clock for ~3.4us.
            if dummy_src is not None:
                for _ in range(n):
                    nc.tensor.transpose(hps[:, NP, :, :], dummy_src[:, 0:128],
                                        idn[:, :])

        for gi, (sl, chunks) in enumerate(TGROUPS):
            dummies(ndum[gi])
            update_c(sl)
            w = sl.stop - sl.start
            tc_t = gates.tile([128, w], F32, tag=f"tc{gi}")
            nc.scalar.activation(tc_t, c_fold[:, sl], AF.Tanh)
            o_t = gates.tile([128, w], F32, tag=f"tc{gi}")
            nc.scalar.activation(o_t, zp["o"][:, sl], AF.Sigmoid, scale=ACT_SCALE)
            nc.vector.tensor_mul(h_fold[:, sl], o_t, tc_t)
            for ci, jj in enumerate(chunks):
                blk = slice(128 * jj, 128 * (jj + 1))
                nc.tensor.transpose(hps[:, jj, :, :], h_fold[:, blk], idn[:, :])
                hview = hps[:, jj, :, :]
                if ci == 0:
                    # critical handoff copy on the Scalar engine
                    nc.scalar.activation(h8s_w[:, jj, :, 64:128], hview,
                                         AF.Copy, scale=S_H)
                else:
                    nc.vector.tensor_scalar_mul(h8s_w[:, jj, :, 64:128], hview, S_H)
                if hT_w is not None:
                    nc.vector.tensor_copy(hT_w[:, jj, :, :], hview)
                if interleave_fn is not None:
                    interleave_fn(jj)

    return tail, h_fold


def _emit_pred_chain(nc, pools, wd_sb, hT_w):
    pd = pools["psum_p"].tile([128, BS], F32, tag="pd")

    def emit_chunk(jj):
        for i in range(2):
            nc.tensor.matmul(
                pd[:, :], wd_sb[:, jj, i, :], hT_w[:, jj, i, :],
                start=(jj == 0 and i == 0), stop=(jj == NP - 1 and i == 1),
            )
    return pd, emit_chunk


def _emit_pred_finish(nc, pools, pd, out_dram, step_idx, predT_w, bd_sb=None):
    pdv = pools["temps"].tile([128, BS], F32, tag="pdv")
    if bd_sb is not None:
        nc.vector.tensor_scalar_add(pdv, pd[:, :], bd_sb[:, 0:1])
    else:
        nc.vector.tensor_copy(pdv[:, :], pd[:, :])
    nc.sync.dma_start(out=out_dram[step_idx], in_=pdv[:, :])
    nc.vector.tensor_copy(predT_w[:, 64:128], pdv[:, :])


def _build(has_b1, has_b2, has_bd, t_steps=T, out_steps=OUT_STEPS):
    nc = bacc.Bacc("TRN2", target_bir_lowering=False, debug=False)

    XT = nc.declare_dram_parameter("xt", [F, t_steps, 192], BF16, isOutput=False)
    W1S = nc.declare_dram_parameter("w1s", [F, G], BF16, isOutput=False)
    U18 = nc.declare_dram_parameter("u18", [128, NP, 2, G], F8, isOutput=False)
    W28 = nc.declare_dram_parameter("w28", [128, NP, 2, G], F8, isOutput=False)
    WD = nc.declare_dram_parameter("wd", [128, NP, 2, F], BF16, isOutput=False)
    IDN = nc.declare_dram_parameter("idn", [128, 128], BF16, isOutput=False)
    if has_b1:
        B1 = nc.declare_dram_parameter("b1f", [4, 128, H], F32, isOutput=False)
    if has_b2:
        B2 = nc.declare_dram_parameter("b2f", [4, 128, H], F32, isOutput=False)
    if has_bd:
        BD = nc.declare_dram_parameter("bdf", [128, 1], F32, isOutput=False)
    OUT = nc.declare_dram_parameter("out", [out_steps, F, BS], F32, isOutput=True)

    with tile.TileContext(nc) as tc, ExitStack() as ctx:
        consts = ctx.enter_context(tc.tile_pool(name="consts", bufs=1))
        pools = {
            "psum": ctx.enter_context(tc.tile_pool(name="psum", bufs=6, space="PSUM")),
            "psum_h": ctx.enter_context(tc.tile_pool(name="psum_h", bufs=1, space="PSUM")),
            "psum_p": ctx.enter_context(tc.tile_pool(name="psum_p", bufs=1, space="PSUM")),
            "gates": ctx.enter_context(tc.tile_pool(name="gates", bufs=8)),
            "temps": ctx.enter_context(tc.tile_pool(name="temps", bufs=4)),
            "hfold": ctx.enter_context(tc.tile_pool(name="hfold", bufs=2)),
        }

        xt_sb = consts.tile([F, t_steps, 192], BF16)
        w1s_sb = consts.tile([F, G], BF16)
        u18_sb = consts.tile([128, NP, 2, G], F8)
        w28_sb = consts.tile([128, NP, 2, G], F8)
        wd_sb = consts.tile([128, NP, 2, F], BF16)
        idn_sb = consts.tile([128, 128], BF16)
        nc.sync.dma_start(out=idn_sb[:], in_=IDN[:])
        nc.sync.dma_start(out=xt_sb[:], in_=XT[:])
        nc.sync.dma_start(out=w1s_sb[:], in_=W1S[:])
        # u18 split by gate-column quarter in chain-consumption order (GORDER)
        # so cell 1's first gate chain starts ~10us in instead of ~40us
        for gate in GORDER:
            off = GATE_OFF[gate]
            nc.sync.dma_start(out=u18_sb[:, :, :, off:off + U],
                              in_=U18[:, :, :, off:off + U])
        nc.sync.dma_start(out=wd_sb[:], in_=WD[:])
        for j in range(NP):  # w28 is first needed at the AR phase (~700us in)
            nc.sync.dma_start(out=w28_sb[:, j, :, :], in_=W28[:, j, :, :])

        b1_tiles = b2_tiles = None
        if has_b1:
            b1_sb = consts.tile([4, 128, H], F32)
            nc.sync.dma_start(out=b1_sb[:], in_=B1[:])
            b1_tiles = {g: b1_sb[i] for i, g in enumerate(("i", "f", "g", "o"))}
        if has_b2:
            b2_sb = consts.tile([4, 128, H], F32)
            nc.sync.dma_start(out=b2_sb[:], in_=B2[:])
            b2_tiles = {g: b2_sb[i] for i, g in enumerate(("i", "f", "g", "o"))}
        bd_sb = None
        if has_bd:
            bd_sb = consts.tile([128, 1], F32)
            nc.sync.dma_start(out=bd_sb[:], in_=BD[:])

        c_fold = consts.tile([128, H], F32)  # persistent cell state

        # double-buffered stationaries; padding columns zeroed once
        h8s_bufs = [consts.tile([128, NP, 2, 192], F8, name=f"h8s{i}") for i in range(2)]
        hT_bufs = [consts.tile([128, NP, 2, BS], BF16, name=f"hT{i}") for i in range(2)]
        predT_bufs = [consts.tile([128, 192], BF16, name=f"pT{i}") for i in range(2)]
        for b in h8s_bufs:
            nc.vector.memset(b[:], 0.0)
        for b in predT_bufs:
            nc.vector.memset(b[:], 0.0)

        def fp8_term(h8s_r, j, w_sb):
            return ("fp8", h8s_r[:, j, :, 0:128], h8s_r[:, j, :, 64:192],
                    lambda off, j=j: w_sb[:, j, :, off:off + H])

        def x_term(t):
            return ("bf16", xt_sb[:, t, 0:128], xt_sb[:, t, 64:192],
                    lambda off: w1s_sb[:, off:off + H])

        def pred_term(predT_r):
            return ("bf16", predT_r[:, 0:128], predT_r[:, 64:192],
                    lambda off: w1s_sb[:, off:off + H])

        WARM_DUM = (4, 3, 2)
        AR_DUM = (9, 6, 6)

        # ---- warmup ----
        cell = 0
        tail = None
        prev_h = None
        for t in range(t_steps):
            h8s_w = h8s_bufs[cell % 2]
            hT_w = hT_bufs[0] if t == t_steps - 1 else None
            if t == 0:
                zp = _emit_chains(nc, pools, [x_term(0)], final=True)
                tail, prev_h = _emit_cell(nc, pools, zp, c_fold, True, idn_sb,
                                          h8s_w, hT_w, b1_tiles)
                cell += 1
                continue
            # pre-start this cell's x chains: they cover the previous tail's
            # latency window on the PE
            zp = _emit_chains(nc, pools, [x_term(t)], final=False)
            tail(None, dummy_src=prev_h, ndum=WARM_DUM)
            h8s_r = h8s_bufs[(cell - 1) % 2]
            terms = [fp8_term(h8s_r, j, u18_sb) for j in range(NP)]
            zp = _emit_chains(nc, pools, terms, zp=zp, final=True)
            tail, prev_h = _emit_cell(nc, pools, zp, c_fold, False, idn_sb,
                                      h8s_w, hT_w, b1_tiles)
            cell += 1
        # ---- pred0 interleaved into the last warmup tail ----
        predT_w = predT_bufs[0]
        pd, pred_chunk = _emit_pred_chain(nc, pools, wd_sb, hT_bufs[0])
        tail(pred_chunk, dummy_src=prev_h, ndum=WARM_DUM)
        _emit_pred_finish(nc, pools, pd, OUT, 0, predT_w, bd_sb)

        # ---- autoregressive ----
        for t in range(out_steps - 1):
            predT_r = predT_bufs[t % 2]
            predT_w = predT_bufs[(t + 1) % 2]
            h8s_r = h8s_bufs[(cell - 1) % 2]
            h8s_w = h8s_bufs[cell % 2]
            terms1 = [fp8_term(h8s_r, j, u18_sb) for j in range(NP)] \
                + [pred_term(predT_r)]
            zp = _emit_chains(nc, pools, terms1, final=True)
            tail1, h1_fold = _emit_cell(nc, pools, zp, c_fold, False, idn_sb,
                                        h8s_w, None, b1_tiles)
            tail1(None, dummy_src=prev_h, ndum=AR_DUM)
            cell += 1
            h8s_r = h8s_bufs[(cell - 1) % 2]
            h8s_w = h8s_bufs[cell % 2]
            hT_w = hT_bufs[cell % 2]
            terms2 = [fp8_term(h8s_r, j, w28_sb) for j in range(NP)]
            zp = _emit_chains(nc, pools, terms2, final=True)
            tail2, prev_h = _emit_cell(nc, pools, zp, c_fold, False, idn_sb,
                                       h8s_w, hT_w, b2_tiles)
            pd, pred_chunk = _emit_pred_chain(nc, pools, wd_sb, hT_w)
            tail2(pred_chunk, dummy_src=h1_fold, ndum=AR_DUM)
            _emit_pred_finish(nc, pools, pd, OUT, t + 1, predT_w, bd_sb)
            cell += 1

    nc.compile()
    return nc


def _fold_bias(b, scale):
    # swapped fold: partitions 64:128 = low half-gate, 0:64 = high half-gate
    out = np.zeros((4, 128, H), np.float32)
    for gi, gname in enumerate(("i", "f", "g", "o")):
        off = GATE_OFF[gname]
        out[gi, 64:128, :] = b[off:off + H][None, :] * scale
        out[gi, 0:64, :] = b[off + H:off + 2 * H][None, :] * scale
    return out


def _prep_fp8_pairs(Wmat, scale):
    """[1024, 4096] -> [128, NP, 2, G] fp8 with plane pairs (j+4, j)."""
    f8 = ml_dtypes.float8_e4m3
    kt = Wmat.reshape(8, 128, G)
    out = np.empty((128, NP, 2, G), np.float32)
    for j in range(NP):
        out[:, j, 0, :] = kt[j + 4]
        out[:, j, 1, :] = kt[j]
    return np.clip(out * scale, -240.0, 240.0).astype(f8)


def kernel(inputs, mean, var, W1, U1, b1, W2, U2, b2, Wd, bd):
    x = np.asarray(inputs, np.float32)
    mean = np.asarray(mean, np.float32)
    var = np.asarray(var, np.float32)
    inv = 1.0 / np.sqrt(var + EPS)
    xn = ((x - mean) * inv - mean) * inv  # reference normalizes twice

    bf = ml_dtypes.bfloat16
    W1 = np.asarray(W1, np.float32)
    U1 = np.asarray(U1, np.float32)
    W2U2 = np.asarray(W2, np.float32) + np.asarray(U2, np.float32)
    Wd = np.asarray(Wd, np.float32)

    wdk = Wd.reshape(8, 128, F)
    wd_sb = np.empty((128, NP, 2, F), np.float32)
    for j in range(NP):
        wd_sb[:, j, 0, :] = wdk[j + 4]
        wd_sb[:, j, 1, :] = wdk[j]

    b1 = np.asarray(b1, np.float32)
    b2 = np.asarray(b2, np.float32)
    bd = np.asarray(bd, np.float32)
    has_b1 = bool(np.any(b1))
    has_b2 = bool(np.any(b2))
    has_bd = bool(np.any(bd))

    key = (has_b1, has_b2, has_bd)
    if key not in _BUILD_CACHE:
        _BUILD_CACHE[key] = _build(*key)
    nc = _BUILD_CACHE[key]

    shared = {
        "w1s": (W1 * S_Z).astype(bf),
        "u18": _prep_fp8_pairs(U1, S_W),
        "w28": _prep_fp8_pairs(W2U2, S_W),
        "wd": wd_sb.astype(bf),
        "idn": np.eye(128, dtype=np.float32).astype(bf),
    }
    if has_b1:
        shared["b1f"] = _fold_bias(b1, S_Z)
    if has_b2:
        shared["b2f"] = _fold_bias(b2, S_Z)
    if has_bd:
        shared["bdf"] = bd.reshape(128, 1).astype(np.float32)

    in_maps = []
    for c in range(NCORES):
        shard = xn[c * BS:(c + 1) * BS]              # [64, 64, 128]
        xtc = np.ascontiguousarray(shard.transpose(2, 1, 0))  # [F, T, BS]
        xt = np.zeros((F, T, 192), np.float32)
        xt[:, :, 64:128] = xtc
        m = dict(shared)
        m["xt"] = xt.astype(bf)
        in_maps.append(m)

    res = run_bass_kernel_spmd(nc, in_maps, core_ids=list(range(NCORES)))
    kernel.last_results = res

    parts = [res.results[c]["out"].transpose(2, 0, 1) for c in range(NCORES)]
    return np.ascontiguousarray(np.concatenate(parts, axis=0), dtype=np.float32)
